# revision 37
# baseline (speedup 1.0000x reference)
"""Trainium2 Bass kernel for FINN-Burger2D flux step (2048x2048, 8 NeuronCores).

Strategy
--------
The per-point MLP a(u) = W3^T tanh(W2^T tanh(W1^T u)) is a smooth odd scalar
function of a scalar.  Computing it exactly costs 64 tanh + ~1100 MACs per
point (~200us/core on ACT) - far beyond the ~12us memory roofline.  Instead we
approximate it with a 3-unit odd basis

    a(u) ~= c0*arctan(a0*u) + c1*tanh(a1*u) + c2*arctan(a2*u)

(max abs error ~1.4e-5 over the input range, refit from the runtime weights at
call time), which the ACT engine evaluates in 3 passes.  The flux combination
collapses (for DX == DY, stencil s0/s1) to

    out = (d + |a|/(2*DX)) * S + (a/(2*DX)) * T
    S = 4*s0*u + s1*(uL+uR+uB+uT),   T = s1*(uL+uB-uR-uT)

S and T are pure linear stencils computed on the TensorEngine with banded
128x128 matrices (row shifts + halo rows via K=2 matmul) and column-shifted
rhs APs.  Work is sharded 256 rows/core across 8 cores; halo rows travel with
each core's input slab, so no collectives are needed.
"""

import numpy as np

import concourse.bass as bass
import concourse.mybir as mybir
import concourse.tile as tile
from concourse.tile import add_dep_helper
from concourse.bass_utils import run_bass_kernel_spmd
from concourse.vector_clock import ScopedClock, VectorClock


def _chunked_drain_and_barrier(self, tick_clock, wait_clock):
    """Tail drain split into <=4-wait chunks (walrus rejects ~11 waits on one
    instruction: 'Too many sync wait commands')."""
    gc = tick_clock.global_clock
    full = list(gc)
    procs = [i for i, t in enumerate(full) if t > 0]
    CHUNK = 1
    for i in range(0, len(procs), CHUNK):
        sub = [0] * len(full)
        for p in procs[i : i + CHUNK]:
            sub[p] = full[p]
        d = self.nc.sync.drain()
        wait_clock.add_sem_waits(d.ins, ScopedClock({None: VectorClock(sub)}))
    # Final drain carries no waits: the serial chain of single-wait drains
    # above already established every proc's tick on SP.
    self.nc.sync.drain()

    self.nc.all_engine_barrier()
    assert self.sems is not None
    popped = self.nc._tile_sem_poison_stack.pop()
    assert popped is self._sem_poison
    self.nc.clear_and_free_semaphores(list(self.sems.allocated().values()))
    self.nc.all_engine_barrier()


tile.TileContext._drain_and_barrier = _chunked_drain_and_barrier

F32 = mybir.dt.float32
F32R = mybir.dt.float32r
AF = mybir.ActivationFunctionType
ALU = mybir.AluOpType

NX = 2048
NY = 2048
DX = 0.01
M = 8                 # cores
RPC = NX // M         # 256 rows per core
P = 128               # partitions
NRB = RPC // P        # row blocks per core (2)
CH = 512              # matmul free-dim chunk (one fp32 PSUM bank)
NCH = NY // CH        # chunks per row (4)

# Fitted offline to the seed-0 reference weights; re-solved (and, if needed,
# re-polished) at runtime from the actual W1/W2/W3 passed in.
# Basis: c0*atan(a0*u) + c1*tanh(a1*u) + cL*u  (the linear term costs no
# ACT pass - it enters the n2 STT directly).
FIT_ALPHAS = (0.91422355, 0.53859007)
FIT_KINDS = ("atan", "tanh")
FIT_C = (-0.54704553, 0.44465964, -0.01491146)

_NP_FUNC = {"atan": np.arctan, "tanh": np.tanh}
_ACT_FUNC = {"atan": AF.Arctan, "tanh": AF.Tanh}


def _mlp_scalar(x, W1, W2, W3):
    h = np.tanh(x[:, None] * W1[0])
    h = np.tanh(h @ W2)
    return (h @ W3)[:, 0]


def _fit_units(W1, W2, W3):
    """Solve the 3-unit approximation for the runtime MLP weights.

    Linear coefficients are re-solved exactly (Lawson-weighted lstsq).  If the
    hardcoded alphas don't reach ~2e-5 max error (weights differ from the
    expected seed), polish alphas with scipy LM.
    """
    xs = np.linspace(0.0, 5.7, 6001)
    fx = _mlp_scalar(xs, W1, W2, W3)

    def basis(al):
        cols = [_NP_FUNC[k](a * xs) for a, k in zip(al, FIT_KINDS)]
        cols.append(xs)
        return np.stack(cols, axis=1)

    def lawson(al, iters=80):
        w = np.ones_like(xs)
        best_m, best_c = np.inf, None
        for _ in range(iters):
            A = basis(al) * w[:, None]
            c, *_ = np.linalg.lstsq(A, fx * w, rcond=None)
            r = basis(al) @ c - fx
            m = float(np.abs(r).max())
            if m < best_m:
                best_m, best_c = m, c.copy()
            w *= np.sqrt(np.abs(r) + 1e-14)
            w /= w.max()
        return best_m, best_c

    al = np.asarray(FIT_ALPHAS, dtype=np.float64)
    m, c = lawson(al)
    if m > 2.5e-4:
        try:
            from scipy.optimize import least_squares

            def cost(la):
                A = basis(np.exp(la))
                cc, *_ = np.linalg.lstsq(A, fx, rcond=None)
                return A @ cc - fx

            sol = least_squares(cost, np.log(al), method="lm", max_nfev=400)
            al2 = np.exp(sol.x)
            m2, c2 = lawson(al2)
            if m2 < m:
                al, m, c = al2, m2, c2
        except Exception:
            pass
    return al, c, m


def _build_consts(s0, s1, fit_c):
    """Packed [128, 768] constant block (all matmul lhsT operands).

    [:,   0:128] TRI : S row stencil  (diag 4*s0, super s1 -> uL, sub s1 -> uR)
    [:, 128:256] BID : T row stencil  (super s1 -> uL, sub -s1 -> uR)
    [:, 256:384] IP  : s1 * I
    [:, 384:512] IN  : -s1 * I
    [0:2,512:640] HS : halo lhsT for S  ([0,0]=s1 top, [1,127]=s1 bottom)
    [0:2,640:768] HT : halo lhsT for T  ([0,0]=s1, [1,127]=-s1)
    """
    tri = np.zeros((P, P), np.float32)
    bid = np.zeros((P, P), np.float32)
    for k in range(P):
        tri[k, k] = 4.0 * s0
        if k + 1 < P:
            tri[k, k + 1] = s1   # out[r] += u[r-1]  (uL)
            bid[k, k + 1] = s1
        if k - 1 >= 0:
            tri[k, k - 1] = s1   # out[r] += u[r+1]  (uR)
            bid[k, k - 1] = -s1
    ip = np.eye(P, dtype=np.float32) * s1
    inn = -ip
    hs = np.zeros((P, P), np.float32)
    ht = np.zeros((P, P), np.float32)
    hs[0, 0] = s1
    hs[1, P - 1] = s1
    ht[0, 0] = s1
    ht[1, P - 1] = -s1
    return np.concatenate([tri, bid, ip, inn, hs, ht], axis=1)


_CACHE = {}
_TRACE_SIM = False
_LAST_TC = [None]


def _build_program(alphas, ratios, d, g, q, repeat=1):
    """Emit the per-core Bass program.

    alphas: ACT input scales for the 3 units
    ratios: (r1, r2) Horner ratios c0/c1, c1/c2
    d:      diffusion coefficient
    g:      c2 / (2*DX)      (signed scale for the a*T term)
    q:      |c2| / (2*DX)    (scale for the |a|*S term)
    repeat: run the whole pipeline this many times (benchmarking variants)
    """
    nc = bass.Bass()
    v = nc.dram_tensor("v", [RPC + 2, NY + 2], F32R, kind="ExternalInput")
    cst = nc.dram_tensor("cst", [P, 768], F32R, kind="ExternalInput")
    outs = [
        [nc.dram_tensor(f"out{rb}_{h}", [P, NY // 2], F32, kind="ExternalOutput")
         for h in range(2)]
        for rb in range(NRB)
    ]

    r1, r2 = ratios
    a1, a2 = alphas

    tc_obj = tile.TileContext(nc, trace_sim=_TRACE_SIM)
    with tc_obj as tc:
        with (
            tc.tile_pool(name="cpool", bufs=1) as cpool,
            tc.tile_pool(name="io", bufs=2) as io,
            tc.tile_pool(name="io1", bufs=1) as io1,
            tc.tile_pool(name="tp3", bufs=2) as tp3,
            tc.tile_pool(name="u4", bufs=4) as u4,
            tc.tile_pool(name="mid", bufs=2) as mid,
            tc.tile_pool(name="oo", bufs=8) as oo,
            tc.tile_pool(name="ps", bufs=4, space="PSUM") as ps,
        ):
            # tiny memset first on the Pool queue so the ACT table warm-up
            # starts at ~0 and the ~1.4us sigmoid_and_others load overlaps
            # the first uc DMA
            wsrc = cpool.tile([1, 16], F32)
            nc.gpsimd.memset(wsrc[:], 0.5)
            warm = cpool.tile([1, 16], F32)
            nc.scalar.activation(warm[:], wsrc[0:1, :], AF.Tanh, scale=1.0)
            # full-width Horner ratio constant for the Pool TT-mult
            r1f = cpool.tile([P, NY], F32)
            nc.gpsimd.memset(r1f[:], float(r1))
            # Pool self-observer for the memset tick
            pscr0 = cpool.tile([1, 1], F32)
            nc.gpsimd.tensor_copy(pscr0[:], r1f[0:1, 0:1])
            c = cpool.tile([P, 768], F32R)
            nc.gpsimd.dma_start(c[:], cst[:, :])
            # PE pre-touch (ldweights: SBUF-read only, no PSUM release chain):
            # absorbs the const-DMA wait so the first real matmul waits only
            # on its own single dependency (1-wait ISA limit).
            nc.tensor.ldweights(c[0:1, 0:2].bitcast(mybir.dt.bfloat16))

            prev_o1 = None
            prev_ot = None

            import contextlib
            loop_cm = (
                tc.For_i(0, repeat, 1, staggered_reset=True,
                         hint_engines=(mybir.EngineType.PE, mybir.EngineType.DVE,
                                       mybir.EngineType.Activation, mybir.EngineType.Pool,
                                       mybir.EngineType.SP))
                if repeat > 1 else contextlib.nullcontext()
            )
            with loop_cm:
              for rb in range(NRB):
                r0 = rb * P
                uc = io1.tile([P, NY + 2], F32R, tag="uc")
                if rb == 0:
                    # split first load: ACT can start on the left half while
                    # the right half is still in flight
                    nc.sync.dma_start(uc[:, 0 : NY // 2 + 2], v[r0 + 1 : r0 + P + 1, 0 : NY // 2 + 2])
                    hh = io.tile([2, NY + 2], F32R, tag="hh")
                    nc.sync.dma_start(hh[:], v[r0 : r0 + P + 2 : P + 1, :])
                    nc.sync.dma_start(uc[:, NY // 2 + 2 :], v[r0 + 1 : r0 + P + 1, NY // 2 + 2 :])
                else:
                    nc.sync.dma_start(uc[:], v[r0 + 1 : r0 + P + 1, :])
                    hh = io.tile([2, NY + 2], F32R, tag="hh")
                    nc.sync.dma_start(hh[:], v[r0 : r0 + P + 2 : P + 1, :])

                ot = io.tile([P, NY], F32, tag="ot")

                if prev_o1 is not None:
                    # PE observer: advances PE's DVE clock past previous
                    # PSUM-release ticks (1-wait ISA limit on matmuls).
                    nc.tensor.ldweights(prev_o1[0:1, 0:1].bitcast(mybir.dt.bfloat16))
                # PE observers of this row block's load lanes.
                nc.tensor.ldweights(uc[0:1, 0:2].bitcast(mybir.dt.bfloat16))
                nc.tensor.ldweights(hh[0:1, 0:2].bitcast(mybir.dt.bfloat16))

                HW = NY // 2
                for h in range(2):
                    hc = slice(1 + h * HW, 1 + (h + 1) * HW)
                    center = uc[:, hc].bitcast(F32)

                    t1 = u4.tile([P, HW], F32, tag="t1")
                    nc.scalar.activation(t1[:], center, _ACT_FUNC[FIT_KINDS[0]], scale=float(a1))
                    t2 = u4.tile([P, HW], F32, tag="t2")
                    nc.scalar.activation(t2[:], center, _ACT_FUNC[FIT_KINDS[1]], scale=float(a2))

                    # n1 = t1*r1 + t2 on Pool (TT pair; STT illegal on Pool),
                    # n2 = n1*r2 + t3 on DVE.
                    pa = u4.tile([P, HW], F32, tag="pa")
                    nc.gpsimd.tensor_mul(pa[:], t1[:], r1f[:, 0:HW])
                    pscr = tp3.tile([1, 1], F32, tag="pscr")
                    nc.gpsimd.tensor_copy(pscr[:], pa[0:1, 0:1])
                    n1 = u4.tile([P, HW], F32, tag="n1")
                    nc.gpsimd.tensor_add(n1[:], pa[:], t2[:])
                    sobn = tp3.tile([1, 1], F32, tag="sobn")
                    nc.vector.tensor_copy(sobn[:], n1[0:1, 0:1])
                    n2 = u4.tile([P, HW], F32, tag="n2")
                    nc.vector.scalar_tensor_tensor(n2[:], n1[:], float(r2), center, ALU.mult, ALU.add)
                    sob2 = tp3.tile([1, 1], F32, tag="sob2")
                    nc.vector.tensor_copy(sob2[:], n2[0:1, 0:1])

                    if prev_ot is not None:
                        sob3 = tp3.tile([1, 1], F32, tag="sob3")
                        nc.vector.tensor_copy(sob3[:], prev_ot[0:1, 0:1])
                        prev_ot = None

                    # ab = |q * n2| on ACT (abs_max is not a legal DVE TS op)
                    ab = u4.tile([P, HW], F32, tag="ab")
                    nc.scalar.activation(ab[:], n2[:], AF.Abs, scale=float(q))
                    sob = tp3.tile([1, 1], F32, tag="sob")
                    nc.vector.tensor_copy(sob[:], ab[0:1, 0:1])

                    for ci in range(HW // CH):
                        c0 = h * HW + ci * CH
                        sp = ps.tile([P, CH], F32, tag="S")
                        nc.tensor.matmul(sp[:], c[:, 0:128], uc[:, c0 + 1 : c0 + CH + 1], start=True, stop=False)
                        nc.tensor.matmul(sp[:], c[:, 256:384], uc[:, c0 : c0 + CH], start=False, stop=False)
                        nc.tensor.matmul(sp[:], c[:, 256:384], uc[:, c0 + 2 : c0 + CH + 2], start=False, stop=False)
                        nc.tensor.matmul(sp[:], c[0:2, 512:640], hh[:, c0 + 1 : c0 + CH + 1], start=False, stop=True)

                        tp = ps.tile([P, CH], F32, tag="T")
                        nc.tensor.matmul(tp[:], c[:, 128:256], uc[:, c0 + 1 : c0 + CH + 1], start=True, stop=False)
                        nc.tensor.matmul(tp[:], c[:, 256:384], uc[:, c0 : c0 + CH], start=False, stop=False)
                        nc.tensor.matmul(tp[:], c[:, 384:512], uc[:, c0 + 2 : c0 + CH + 2], start=False, stop=False)
                        nc.tensor.matmul(tp[:], c[0:2, 640:768], hh[:, c0 + 1 : c0 + CH + 1], start=False, stop=True)

                        ls = slice(ci * CH, (ci + 1) * CH)
                        o2 = oo.tile([P, CH], F32, tag="o2")
                        nc.vector.scalar_tensor_tensor(o2[:], n2[:, ls], float(g), tp[:], ALU.mult, ALU.mult)
                        o1 = oo.tile([P, CH], F32, tag="o1")
                        nc.vector.scalar_tensor_tensor(o1[:], ab[:, ls], float(d), sp[:], ALU.add, ALU.mult)
                        nc.gpsimd.tensor_add(ot[:, c0 : c0 + CH], o1[:], o2[:])
                        prev_o1 = o1

                    nc.sync.dma_start(outs[rb][h][:, :], ot[:, h * HW : (h + 1) * HW])
                prev_ot = ot
    _LAST_TC[0] = tc_obj
    return nc


def kernel(u, W1, W2, W3, D, BC, stencil):
    u = np.ascontiguousarray(u, dtype=np.float32)
    W1 = np.asarray(W1, dtype=np.float32)
    W2 = np.asarray(W2, dtype=np.float32)
    W3 = np.asarray(W3, dtype=np.float32)
    d = float(np.asarray(D).ravel()[0])
    bc0 = float(np.asarray(BC)[0, 0])
    bc1 = float(np.asarray(BC)[1, 0])
    s0 = float(np.asarray(stencil)[0])
    s1 = float(np.asarray(stencil)[1])

    al, cc, _ = _fit_units(W1, W2, W3)
    r1 = cc[0] / cc[1]
    r2 = cc[1] / cc[2]
    g = cc[2] / (2.0 * DX)
    q = abs(cc[2]) / (2.0 * DX)

    key = (tuple(np.round(al, 10)), round(r1, 10), round(r2, 10),
           round(d, 12), round(g, 10), round(q, 10))
    if key not in _CACHE:
        _CACHE.clear()
        _CACHE[key] = _build_program(al, (r1, r2), d, g, q)
    nc = _CACHE[key]

    # Padded slab: vpad[i, j] = u[i-1, j-1]; boundary fills per the reference
    # (row -1 / col -1 -> bc0, row NX / col NY -> bc1).
    vpad = np.empty((NX + 2, NY + 2), dtype=np.float32)
    vpad[1:-1, 1:-1] = u
    vpad[0, :] = bc0
    vpad[-1, :] = bc1
    vpad[:, 0] = bc0
    vpad[:, -1] = bc1

    cst = _build_consts(s0, s1, cc)

    in_maps = []
    for k in range(M):
        r0 = k * RPC
        in_maps.append({"v": np.ascontiguousarray(vpad[r0 : r0 + RPC + 2, :]),
                        "cst": cst})

    res = run_bass_kernel_spmd(nc, in_maps, core_ids=list(range(M)))
    full = np.empty((NX, NY), dtype=np.float32)
    for k in range(M):
        r = res.results[k]
        row0 = k * RPC
        full[row0 : row0 + P, :] = r["out0"]
        full[row0 + P : row0 + 2 * P, 0 : NY // 2] = r["out1_0"]
        full[row0 + P : row0 + 2 * P, NY // 2 :] = r["out1_1"]
    return full
